# revision 19
# baseline (speedup 1.0000x reference)
"""ArcLengthLoss distributed Bass kernel for 8 TRN2 NeuronCores (v7).

Reference computation:
    s = output[:, :, 0]                               # [32, 153]
    A = s[:, a1] - s[:, a2]; a_term = exp(A.mean(1))  # [32]
    b1 = s[:, direct]                                 # [32, NC]
    b2 = sum_l mask(l<seg_len) * s[:, pad_idx[:, l]]  # [32, NC]
    loss = (a_term + |b1-b2|.mean(1)).mean()

The per-combo gather/sum is a matmul against a signed indicator matrix
W[sec, combo], shipped dense in fp8 (0/±1 exact in e4m3).  Sections are
usage-ranked on the partition dim and combos are packed into tiles
sorted by the highest rank they touch.  Tiles needing <=64 rows are
PAIRED two-per-transfer at partition floors 0/64 (the floor-1 matmul
runs at tile_position (64, 32a) against a host-shifted copy of s —
verified bit-exact), so every DMA still spans all 128 partitions: the
DMA engines are assigned by destination partition, and short transfers
collapse onto 3 of 16 engines (measured 3x slowdown).  W streams in
runs alternating the sync HWDGE queue and the gpsimd SWDGE queue with a
unique SBUF buffer per run; the scalar queue only carries the small
SP/WA/WH inputs and the result.  The 25 rarest sections ride a
zero-padded K=128 "hi" strip chain-accumulated on the last two tiles.
PSUM drains: scalar ACT Abs(+accum) for 5 groups incl. the last,
two-pass Vector (negate-to-bf16 + scalar_tensor_tensor max w/
accumulate) for 3.  tensor_tensor_reduce is avoided (crashes the exec
unit); DoubleRow fp8 was rejected (ISA pins its output to PSUM
partition 0, killing the 4-way packing).  Combos the dense build cannot
express exactly (duplicate targets -> |entry| past fp8 integer range,
or hi-capacity overflow) are computed on the host (0 for the reference
tables).
"""
import sys

if "/opt/trn_rl_repo" not in sys.path:
    sys.path.insert(0, "/opt/trn_rl_repo")

import numpy as np
import ml_dtypes

import concourse.bass as bass  # noqa: F401
import concourse.bacc as bacc
import concourse.tile as tile
from concourse import mybir
from concourse.bass_utils import run_bass_kernel_spmd

# ---- problem constants ----
B = 32
S = 153
L = 17
NA = 136
CORES = 8
TILE = 1024
T = 32                    # tiles per core
PERCORE = T * TILE
NTOT = PERCORE * CORES

N_HI_TILES = 2
HI_TILES = [30, 31]
HI_CAP = N_HI_TILES * TILE * CORES
SCALAR_DRAINS = (0, 2, 4, 5, 6, 7)

_DT = mybir.dt
_CACHE = {}


def _transfer_runs(nx):
    """Mega-DMA runs over transfer indices: two singles first so matmuls
    start early, fours in the middle, small tail."""
    runs = [(0, 1), (1, 1)]
    x = 2
    while nx - x > 5:
        runs.append((x, 4))
        x += 4
    while nx - x > 0:
        k = min(2, nx - x)
        runs.append((x, k))
        x += k
    return runs


def build_nc(npairs):
    nx = T - npairs
    runs = _transfer_runs(nx)

    def tile_of_x(x):
        return 2 * x if x < npairs else npairs + x

    nc = bacc.Bacc("TRN2", target_bir_lowering=False, debug=False,
                   num_devices=CORES)

    sp_d = nc.dram_tensor("SP", [128, 3 * B], _DT.float32,
                          kind="ExternalInput")
    wa_d = nc.dram_tensor("WA", [128, 2 * NA], _DT.int16,
                          kind="ExternalInput")
    wd_d = nc.dram_tensor("WD", [128, nx * TILE], _DT.int8,
                          kind="ExternalInput")
    wh_d = nc.dram_tensor("WH", [128, N_HI_TILES * TILE], _DT.int8,
                          kind="ExternalInput")
    o_d = nc.dram_tensor("outv", [128, 2], _DT.float32, kind="ExternalOutput")

    TT = mybir.AluOpType

    with tile.TileContext(nc) as tc:
        with (
            tc.tile_pool(name="const", bufs=1) as cpool,
            tc.tile_pool(name="wts", bufs=1) as wpool,
            tc.tile_pool(name="drain", bufs=3) as dpool,
            tc.tile_pool(name="psum", bufs=3, space="PSUM") as ppool,
            tc.tile_pool(name="psumA", bufs=1, space="PSUM") as papool,
        ):
            # ---- small input DMAs.  SP leads the sync queue so the
            # fp8 casts (which gate every matmul) start earliest; WA/WH
            # ride the otherwise-idle scalar queue.  WH ships full-size
            # zero-padded from the host (0 x NaN garbage would poison
            # PSUM, and memsets cost engine time).
            sp_f = cpool.tile([128, 3 * B], _DT.float32)
            nc.sync.dma_start(sp_f[:], sp_d.ap())
            wa_sb = cpool.tile([128, 2 * NA], _DT.int16)
            wh_sb = cpool.tile([128, N_HI_TILES * TILE], _DT.int8)

            # ---- converts (vector) — s8 casts first, they gate matmuls
            s8_lo = cpool.tile([128, B], _DT.float8e4)
            nc.vector.tensor_copy(s8_lo[:], sp_f[:, 0:B])
            s8_f1 = cpool.tile([128, B], _DT.float8e4)
            nc.vector.tensor_copy(s8_f1[:], sp_f[:, 2 * B:3 * B])
            s8_hi = cpool.tile([128, B], _DT.float8e4)
            nc.vector.memset(s8_hi[:], 0.0)
            nc.vector.tensor_copy(s8_hi[0:32, :], sp_f[0:32, B:2 * B])
            sT_lo = cpool.tile([128, B], _DT.bfloat16)
            nc.vector.tensor_copy(sT_lo[:], sp_f[:, 0:B])
            sT_hi = cpool.tile([32, B], _DT.bfloat16)
            nc.vector.tensor_copy(sT_hi[:], sp_f[0:32, B:2 * B])

            wh8 = wh_sb[:].bitcast(_DT.float8e4).rearrange(
                "p (ht c) -> p ht c", ht=N_HI_TILES)
            wa16 = wa_sb[:].bitcast(_DT.bfloat16)

            # ---- W stream: every mega has its own SBUF buffer, so all
            # triggers fire upfront and the two queues stream while the
            # PE consumes in order.
            bacc_t = cpool.tile([128, T // 4 + 1], _DT.float32)
            st = [None] * T
            ps_roll = {}
            for ri, (x0, k) in enumerate(runs):
                mega = wpool.tile([128, k * TILE], _DT.int8,
                                  tag=f"mega_{x0}", name=f"mega_{x0}")
                if ri < 2:
                    eng = nc.sync
                else:
                    eng = [nc.scalar, nc.gpsimd, nc.sync][(ri - 2) % 3]
                eng.dma_start(mega[:], wd_d.ap()[:, x0 * TILE:(x0 + k) * TILE])
                for i in range(k):
                    x = x0 + i
                    tt = tile_of_x(x)
                    if x < npairs:
                        st[tt] = (mega, i, "f0")
                        st[tt + 1] = (mega, i, "f1")
                    else:
                        st[tt] = (mega, i, "full")
                if ri == 2:
                    # WA/WH slot in behind scalar's first mega: early
                    # enough for the A-term and hi tiles, without
                    # delaying the first W transfers
                    nc.scalar.dma_start(wa_sb[:], wa_d.ap())
                    nc.scalar.dma_start(wh_sb[:], wh_d.ap())

            def s_load(t):
                pass

            def s_mm(t):
                a = t % 4
                if a == 0:
                    ps_roll["ps"] = ppool.tile([128, 1024], _DT.float32,
                                               tag="ps", name=f"ps_{t}")
                    ps_roll[t // 4] = ps_roll["ps"]
                psum = ps_roll["ps"]
                mega, i, kind = st[t]
                w8 = mega[:].bitcast(_DT.float8e4)
                off = i * TILE
                hi = t in HI_TILES
                for q in range(2):
                    sub = psum[32 * a:32 * (a + 1), 512 * q:512 * (q + 1)]
                    cols = slice(off + 512 * q, off + 512 * (q + 1))
                    if kind == "f0":
                        nc.tensor.matmul(
                            sub, s8_lo[0:64, :], w8[0:64, cols],
                            start=True, stop=not hi,
                            skip_group_check=True, tile_position=(0, 32 * a))
                    elif kind == "f1":
                        nc.tensor.matmul(
                            sub, s8_f1[64:128, :], w8[64:128, cols],
                            start=True, stop=not hi,
                            skip_group_check=True, tile_position=(64, 32 * a))
                    else:
                        nc.tensor.matmul(
                            sub, s8_lo[:], w8[:, cols],
                            start=True, stop=not hi,
                            skip_group_check=True, tile_position=(0, 32 * a))
                    if hi:
                        ht = HI_TILES.index(t)
                        nc.tensor.matmul(
                            sub, s8_hi[:],
                            wh8[:, ht, 512 * q:512 * (q + 1)],
                            start=False, stop=True,
                            skip_group_check=True, tile_position=(0, 32 * a))

            def s_drain(t):
                if t % 4 != 3:
                    return
                g = t // 4
                psum = ps_roll.pop(g)
                trash = dpool.tile([128, 1024], _DT.bfloat16,
                                   tag="trash", name=f"trash_{t}")
                if g == T // 4 - 1:
                    # last group: halve across scalar and vector to cut
                    # the serial tail
                    nc.scalar.activation(
                        trash[:, 0:512], psum[:, 0:512],
                        mybir.ActivationFunctionType.Abs,
                        accum_out=bacc_t[:, g:g + 1])
                    ng = dpool.tile([128, 1024], _DT.bfloat16,
                                    tag="ng", name=f"ng_{t}")
                    nc.vector.tensor_scalar(ng[:, 512:1024],
                                            psum[:, 512:1024], -1.0, None,
                                            op0=TT.mult)
                    nc.vector.scalar_tensor_tensor(
                        trash[:, 512:1024], ng[:, 512:1024], -1.0,
                        ng[:, 512:1024],
                        op0=TT.mult, op1=TT.max,
                        accum_out=bacc_t[:, g + 1:g + 2])
                elif g in SCALAR_DRAINS:
                    nc.scalar.activation(
                        trash[:], psum[:],
                        mybir.ActivationFunctionType.Abs,
                        accum_out=bacc_t[:, g:g + 1])
                else:
                    ng = dpool.tile([128, 1024], _DT.bfloat16,
                                    tag="ng", name=f"ng_{t}")
                    nc.vector.tensor_scalar(ng[:], psum[:], -1.0, None,
                                            op0=TT.mult)
                    nc.vector.scalar_tensor_tensor(
                        trash[:], ng[:], -1.0, ng[:],
                        op0=TT.mult, op1=TT.max,
                        accum_out=bacc_t[:, g:g + 1])
                st[t] = None

            asum = cpool.tile([B, 1], _DT.float32)

            def emit_a_term():
                psa = papool.tile([B, NA], _DT.float32, tag="psa")
                nc.tensor.matmul(psa[:], sT_lo[:], wa16[:, 0:NA],
                                 start=True, stop=False)
                nc.tensor.matmul(psa[:], sT_hi[:], wa16[0:32, NA:2 * NA],
                                 start=False, stop=True)
                nc.vector.tensor_reduce(asum[:], psa[:],
                                        axis=mybir.AxisListType.X,
                                        op=mybir.AluOpType.add)

            def s_nop(t):
                pass

            stages = [s_load, s_nop, s_nop, s_mm, s_drain]
            NS = len(stages)
            for step in range(T + NS - 1):
                for si in reversed(range(NS)):
                    t = step - si
                    if 0 <= t < T:
                        stages[si](t)
                if step == 14:
                    emit_a_term()

            outv = cpool.tile([128, 2], _DT.float32)
            nc.vector.memset(outv[:], 0.0)
            nc.vector.tensor_reduce(outv[:, 0:1], bacc_t[:],
                                    axis=mybir.AxisListType.X,
                                    op=mybir.AluOpType.add)
            nc.vector.tensor_copy(outv[0:B, 1:2], asum[:])
            nc.scalar.dma_start(o_d.ap(), outv[:])

    nc.compile()
    return nc


def prepare(inputs):
    """Host-side prep: rank sections, sort combos by max rank, pair
    short tiles, build device arrays."""
    s = np.asarray(inputs["output"], np.float32)[:, :, 0]
    a1 = np.asarray(inputs["a1"], np.int64)
    a2 = np.asarray(inputs["a2"], np.int64)
    direct = np.asarray(inputs["direct"], np.int64)
    pad = np.asarray(inputs["pad_idx"], np.int64)
    seg = np.asarray(inputs["seg_len"], np.int64)
    NCv = direct.shape[0]
    lane = np.arange(L)[None, :]
    act = lane < seg[:, None]

    padrefs = np.bincount(pad[act], minlength=S)
    dirrefs = np.bincount(direct, minlength=S)
    usage = padrefs + dirrefs
    order = np.argsort(-usage, kind="stable")
    rank = np.empty(S, np.int64)
    rank[order] = np.arange(S)

    # duplicate targets within a combo could push |W entry| past the fp8
    # exact-integer range -> host
    a_ = np.where(act, pad, 2000 + lane)
    tcat = np.concatenate([np.where(direct < S, direct, 3000)[:, None], a_], 1)
    tcat.sort(axis=1)
    host = (tcat[:, 1:] == tcat[:, :-1]).any(1)

    rd = rank[direct]
    rp = np.where(act, rank[pad], 0)
    maxrank = np.maximum(rd, rp.max(1))
    hi_idx = np.flatnonzero((maxrank >= 128) & ~host)
    if hi_idx.size > HI_CAP:
        host[hi_idx[HI_CAP:]] = True

    dev_sorted = np.flatnonzero(~host)
    dev_sorted = dev_sorted[np.argsort(maxrank[dev_sorted], kind="stable")]
    assert dev_sorted.size <= NTOT, "combo overflow"
    core_of = np.full(NCv, -1, np.int64)
    tile_of = np.full(NCv, -1, np.int64)
    col_of = np.full(NCv, -1, np.int64)
    seqpos = np.arange(dev_sorted.size)
    core_of[dev_sorted] = seqpos % CORES
    pos = seqpos // CORES
    tile_of[dev_sorted] = pos // TILE
    col_of[dev_sorted] = pos % TILE

    # per-tile max rank -> pair count (adjacent tiles both <=64 rows;
    # the hi tiles are never paired)
    r_t = np.zeros(T, np.int64)
    mr = np.minimum(maxrank[dev_sorted], 127)
    np.maximum.at(r_t, tile_of[dev_sorted], mr + 1)
    npairs = 0
    while (2 * npairs + 1 < min(HI_TILES) and r_t[2 * npairs] <= 64
           and r_t[2 * npairs + 1] <= 64):
        npairs += 1
    _CACHE["npairs"] = npairs
    nx = T - npairs

    dev = dev_sorted
    pr, pl = np.nonzero(act[dev])
    e_combo = np.concatenate([dev[pr], dev])
    e_row = rank[np.concatenate([pad[dev[pr], pl], direct[dev]])]
    e_val = np.concatenate([np.full(pr.size, -1.0, np.float32),
                            np.full(dev.size, 1.0, np.float32)])
    e_core = core_of[e_combo]
    e_tile = tile_of[e_combo]
    e_col = col_of[e_combo]

    hi_pos_arr = np.full(T, -1, np.int64)
    for i, t_ in enumerate(HI_TILES):
        hi_pos_arr[t_] = i

    WDf = np.zeros((CORES, 128, T, TILE), np.float32)
    lo_e = np.flatnonzero(e_row < 128)
    np.add.at(WDf, (e_core[lo_e], e_row[lo_e], e_tile[lo_e], e_col[lo_e]),
              e_val[lo_e])
    WHf = np.zeros((CORES, 32, N_HI_TILES, TILE), np.float32)
    he = np.flatnonzero(e_row >= 128)
    if he.size:
        np.add.at(WHf, (e_core[he], e_row[he] - 128,
                        hi_pos_arr[e_tile[he]], e_col[he]), e_val[he])
    WD8 = WDf.astype(ml_dtypes.float8_e4m3)
    # transfer-major layout: pairs stack two tiles at partition floors
    WDX = np.zeros((CORES, 128, nx, TILE), ml_dtypes.float8_e4m3)
    for x in range(nx):
        if x < npairs:
            WDX[:, 0:64, x] = WD8[:, 0:64, 2 * x]
            WDX[:, 64:128, x] = WD8[:, 0:64, 2 * x + 1]
        else:
            WDX[:, :, x] = WD8[:, :, npairs + x]
    WD = WDX.view(np.int8).reshape(CORES, 128, nx * TILE)
    WHp = np.zeros((CORES, 128, N_HI_TILES, TILE), ml_dtypes.float8_e4m3)
    WHp[:, 0:32] = WHf.astype(ml_dtypes.float8_e4m3)
    WH = WHp.view(np.int8).reshape(CORES, 128, N_HI_TILES * TILE)

    sTa = np.zeros((160, B), np.float32)
    sTa[rank] = s.T
    SP = np.zeros((128, 3 * B), np.float32)
    SP[:, 0:B] = sTa[0:128]
    SP[0:32, B:2 * B] = sTa[128:160]
    SP[64:128, 2 * B:3 * B] = sTa[0:64]

    r1 = rank[a1]
    r2 = rank[a2]
    WAf = np.zeros((128, 2 * NA), np.float32)
    i_lo1 = np.flatnonzero(r1 < 128)
    np.add.at(WAf, (r1[i_lo1], i_lo1), 1.0)
    i_lo2 = np.flatnonzero(r2 < 128)
    np.add.at(WAf, (r2[i_lo2], i_lo2), -1.0)
    i_hi1 = np.flatnonzero(r1 >= 128)
    np.add.at(WAf, (r1[i_hi1] - 128, NA + i_hi1), 1.0)
    i_hi2 = np.flatnonzero(r2 >= 128)
    np.add.at(WAf, (r2[i_hi2] - 128, NA + i_hi2), -1.0)
    WA = WAf.astype(ml_dtypes.bfloat16).view(np.int16)

    hs = np.flatnonzero(host)
    host_abs = 0.0
    if hs.size:
        m = act[hs].astype(np.float32)
        b2 = np.einsum("bnl,nl->bn", s[:, pad[hs]], m)
        b1 = s[:, direct[hs]]
        host_abs = float(np.abs(b1 - b2).sum())

    in_maps = []
    for c in range(CORES):
        in_maps.append({"SP": SP, "WA": WA, "WD": WD[c], "WH": WH[c]})
    return in_maps, dict(NCv=NCv, host_abs=host_abs, n_host=int(hs.size))


def combine(outs, meta):
    total_abs = meta["host_abs"] + sum(float(outs[i]["outv"][:, 0].sum())
                                       for i in range(CORES))
    mean_a = float(np.exp(outs[0]["outv"][0:B, 1] / NA).mean())
    val = mean_a + total_abs / (B * meta["NCv"])
    return np.asarray(val, dtype=np.float32)


def get_nc():
    npairs = _CACHE.get("npairs", 0)
    key = ("nc", npairs)
    if key not in _CACHE:
        _CACHE[key] = build_nc(npairs)
    return _CACHE[key]


def kernel(**inputs) -> np.ndarray:
    in_maps, meta = prepare(inputs)
    res = run_bass_kernel_spmd(get_nc(), in_maps, core_ids=list(range(CORES)))
    return combine(res.results, meta)


# revision 20
# speedup vs baseline: 1.0410x; 1.0410x over previous
"""ArcLengthLoss distributed Bass kernel for 8 TRN2 NeuronCores (v7).

Reference computation:
    s = output[:, :, 0]                               # [32, 153]
    A = s[:, a1] - s[:, a2]; a_term = exp(A.mean(1))  # [32]
    b1 = s[:, direct]                                 # [32, NC]
    b2 = sum_l mask(l<seg_len) * s[:, pad_idx[:, l]]  # [32, NC]
    loss = (a_term + |b1-b2|.mean(1)).mean()

The per-combo gather/sum is a matmul against a signed indicator matrix
W[sec, combo], shipped dense in fp8 (0/±1 exact in e4m3).  Sections are
usage-ranked on the partition dim and combos are packed into tiles
sorted by the highest rank they touch.  Tiles needing <=64 rows are
PAIRED two-per-transfer at partition floors 0/64 (the floor-1 matmul
runs at tile_position (64, 32a) against a host-shifted copy of s —
verified bit-exact), so every DMA still spans all 128 partitions: the
DMA engines are assigned by destination partition, and short transfers
collapse onto 3 of 16 engines (measured 3x slowdown).  W streams in
runs alternating the sync HWDGE queue and the gpsimd SWDGE queue with a
unique SBUF buffer per run; the scalar queue only carries the small
SP/WA/WH inputs and the result.  The 25 rarest sections ride a
zero-padded K=128 "hi" strip chain-accumulated on the last two tiles.
PSUM drains: scalar ACT Abs(+accum) for 5 groups incl. the last,
two-pass Vector (negate-to-bf16 + scalar_tensor_tensor max w/
accumulate) for 3.  tensor_tensor_reduce is avoided (crashes the exec
unit); DoubleRow fp8 was rejected (ISA pins its output to PSUM
partition 0, killing the 4-way packing).  Combos the dense build cannot
express exactly (duplicate targets -> |entry| past fp8 integer range,
or hi-capacity overflow) are computed on the host (0 for the reference
tables).
"""
import sys

if "/opt/trn_rl_repo" not in sys.path:
    sys.path.insert(0, "/opt/trn_rl_repo")

import numpy as np
import ml_dtypes

import concourse.bass as bass  # noqa: F401
import concourse.bacc as bacc
import concourse.tile as tile
from concourse import mybir
from concourse.bass_utils import run_bass_kernel_spmd

# ---- problem constants ----
B = 32
S = 153
L = 17
NA = 136
CORES = 8
TILE = 1024
T = 32                    # tiles per core
PERCORE = T * TILE
NTOT = PERCORE * CORES

N_HI_TILES = 2
HI_TILES = [30, 31]
HI_CAP = N_HI_TILES * TILE * CORES
SCALAR_DRAINS = (0, 2, 4, 5, 6, 7)

_DT = mybir.dt
_CACHE = {}


def _transfer_runs(nx):
    """Mega-DMA runs over transfer indices: pairs, round-robined across
    the three queues in need order so the delivery frontier tracks the
    PE's consumption order."""
    runs = []
    x = 0
    while nx - x > 0:
        k = min(2, nx - x)
        runs.append((x, k))
        x += k
    return runs


def build_nc(npairs):
    nx = T - npairs
    runs = _transfer_runs(nx)

    def tile_of_x(x):
        return 2 * x if x < npairs else npairs + x

    nc = bacc.Bacc("TRN2", target_bir_lowering=False, debug=False,
                   num_devices=CORES)

    sp_d = nc.dram_tensor("SP", [128, 3 * B], _DT.float32,
                          kind="ExternalInput")
    wa_d = nc.dram_tensor("WA", [128, 2 * NA], _DT.int16,
                          kind="ExternalInput")
    wd_d = nc.dram_tensor("WD", [128, nx * TILE], _DT.int8,
                          kind="ExternalInput")
    wh_d = nc.dram_tensor("WH", [128, N_HI_TILES * TILE], _DT.int8,
                          kind="ExternalInput")
    o_d = nc.dram_tensor("outv", [128, 2], _DT.float32, kind="ExternalOutput")

    TT = mybir.AluOpType

    with tile.TileContext(nc) as tc:
        with (
            tc.tile_pool(name="const", bufs=1) as cpool,
            tc.tile_pool(name="wts", bufs=1) as wpool,
            tc.tile_pool(name="drain", bufs=3) as dpool,
            tc.tile_pool(name="psum", bufs=3, space="PSUM") as ppool,
            tc.tile_pool(name="psumA", bufs=1, space="PSUM") as papool,
        ):
            # ---- small input DMAs.  SP leads the sync queue so the
            # fp8 casts (which gate every matmul) start earliest; WA/WH
            # ride the otherwise-idle scalar queue.  WH ships full-size
            # zero-padded from the host (0 x NaN garbage would poison
            # PSUM, and memsets cost engine time).
            sp_f = cpool.tile([128, 3 * B], _DT.float32)
            nc.sync.dma_start(sp_f[:], sp_d.ap())
            wa_sb = cpool.tile([128, 2 * NA], _DT.int16)
            wh_sb = cpool.tile([128, N_HI_TILES * TILE], _DT.int8)

            # ---- converts (vector) — s8 casts first, they gate matmuls
            s8_lo = cpool.tile([128, B], _DT.float8e4)
            nc.vector.tensor_copy(s8_lo[:], sp_f[:, 0:B])
            s8_f1 = cpool.tile([128, B], _DT.float8e4)
            nc.vector.tensor_copy(s8_f1[:], sp_f[:, 2 * B:3 * B])
            s8_hi = cpool.tile([128, B], _DT.float8e4)
            nc.vector.memset(s8_hi[:], 0.0)
            nc.vector.tensor_copy(s8_hi[0:32, :], sp_f[0:32, B:2 * B])
            sT_lo = cpool.tile([128, B], _DT.bfloat16)
            nc.vector.tensor_copy(sT_lo[:], sp_f[:, 0:B])
            sT_hi = cpool.tile([32, B], _DT.bfloat16)
            nc.vector.tensor_copy(sT_hi[:], sp_f[0:32, B:2 * B])

            wh8 = wh_sb[:].bitcast(_DT.float8e4).rearrange(
                "p (ht c) -> p ht c", ht=N_HI_TILES)
            wa16 = wa_sb[:].bitcast(_DT.bfloat16)

            # ---- W stream: every mega has its own SBUF buffer, so all
            # triggers fire upfront and the two queues stream while the
            # PE consumes in order.
            bacc_t = cpool.tile([128, T // 4 + 1], _DT.float32)
            st = [None] * T
            ps_roll = {}
            for ri, (x0, k) in enumerate(runs):
                mega = wpool.tile([128, k * TILE], _DT.int8,
                                  tag=f"mega_{x0}", name=f"mega_{x0}")
                eng = [nc.sync, nc.scalar, nc.gpsimd][ri % 3]
                eng.dma_start(mega[:], wd_d.ap()[:, x0 * TILE:(x0 + k) * TILE])
                for i in range(k):
                    x = x0 + i
                    tt = tile_of_x(x)
                    if x < npairs:
                        st[tt] = (mega, i, "f0")
                        st[tt + 1] = (mega, i, "f1")
                    else:
                        st[tt] = (mega, i, "full")
                if ri == 1:
                    # WA/WH slot in behind scalar's first mega: early
                    # enough for the A-term and hi tiles, without
                    # delaying the first W transfers
                    nc.scalar.dma_start(wa_sb[:], wa_d.ap())
                    nc.scalar.dma_start(wh_sb[:], wh_d.ap())

            def s_load(t):
                pass

            def s_mm(t):
                a = t % 4
                if a == 0:
                    ps_roll["ps"] = ppool.tile([128, 1024], _DT.float32,
                                               tag="ps", name=f"ps_{t}")
                    ps_roll[t // 4] = ps_roll["ps"]
                psum = ps_roll["ps"]
                mega, i, kind = st[t]
                w8 = mega[:].bitcast(_DT.float8e4)
                off = i * TILE
                hi = t in HI_TILES
                for q in range(2):
                    sub = psum[32 * a:32 * (a + 1), 512 * q:512 * (q + 1)]
                    cols = slice(off + 512 * q, off + 512 * (q + 1))
                    if kind == "f0":
                        nc.tensor.matmul(
                            sub, s8_lo[0:64, :], w8[0:64, cols],
                            start=True, stop=not hi,
                            skip_group_check=True, tile_position=(0, 32 * a))
                    elif kind == "f1":
                        nc.tensor.matmul(
                            sub, s8_f1[64:128, :], w8[64:128, cols],
                            start=True, stop=not hi,
                            skip_group_check=True, tile_position=(64, 32 * a))
                    else:
                        nc.tensor.matmul(
                            sub, s8_lo[:], w8[:, cols],
                            start=True, stop=not hi,
                            skip_group_check=True, tile_position=(0, 32 * a))
                    if hi:
                        ht = HI_TILES.index(t)
                        nc.tensor.matmul(
                            sub, s8_hi[:],
                            wh8[:, ht, 512 * q:512 * (q + 1)],
                            start=False, stop=True,
                            skip_group_check=True, tile_position=(0, 32 * a))

            def s_drain(t):
                if t % 4 != 3:
                    return
                g = t // 4
                psum = ps_roll.pop(g)
                trash = dpool.tile([128, 1024], _DT.bfloat16,
                                   tag="trash", name=f"trash_{t}")
                if g == T // 4 - 1:
                    # last group: halve across scalar and vector to cut
                    # the serial tail
                    nc.scalar.activation(
                        trash[:, 0:512], psum[:, 0:512],
                        mybir.ActivationFunctionType.Abs,
                        accum_out=bacc_t[:, g:g + 1])
                    ng = dpool.tile([128, 1024], _DT.bfloat16,
                                    tag="ng", name=f"ng_{t}")
                    nc.vector.tensor_scalar(ng[:, 512:1024],
                                            psum[:, 512:1024], -1.0, None,
                                            op0=TT.mult)
                    nc.vector.scalar_tensor_tensor(
                        trash[:, 512:1024], ng[:, 512:1024], -1.0,
                        ng[:, 512:1024],
                        op0=TT.mult, op1=TT.max,
                        accum_out=bacc_t[:, g + 1:g + 2])
                elif g in SCALAR_DRAINS:
                    nc.scalar.activation(
                        trash[:], psum[:],
                        mybir.ActivationFunctionType.Abs,
                        accum_out=bacc_t[:, g:g + 1])
                else:
                    ng = dpool.tile([128, 1024], _DT.bfloat16,
                                    tag="ng", name=f"ng_{t}")
                    nc.vector.tensor_scalar(ng[:], psum[:], -1.0, None,
                                            op0=TT.mult)
                    nc.vector.scalar_tensor_tensor(
                        trash[:], ng[:], -1.0, ng[:],
                        op0=TT.mult, op1=TT.max,
                        accum_out=bacc_t[:, g:g + 1])
                st[t] = None

            asum = cpool.tile([B, 1], _DT.float32)

            def emit_a_term():
                psa = papool.tile([B, NA], _DT.float32, tag="psa")
                nc.tensor.matmul(psa[:], sT_lo[:], wa16[:, 0:NA],
                                 start=True, stop=False)
                nc.tensor.matmul(psa[:], sT_hi[:], wa16[0:32, NA:2 * NA],
                                 start=False, stop=True)
                nc.vector.tensor_reduce(asum[:], psa[:],
                                        axis=mybir.AxisListType.X,
                                        op=mybir.AluOpType.add)

            def s_nop(t):
                pass

            stages = [s_load, s_nop, s_nop, s_mm, s_drain]
            NS = len(stages)
            for step in range(T + NS - 1):
                for si in reversed(range(NS)):
                    t = step - si
                    if 0 <= t < T:
                        stages[si](t)
                if step == 14:
                    emit_a_term()

            outv = cpool.tile([128, 2], _DT.float32)
            nc.vector.memset(outv[:], 0.0)
            nc.vector.tensor_reduce(outv[:, 0:1], bacc_t[:],
                                    axis=mybir.AxisListType.X,
                                    op=mybir.AluOpType.add)
            nc.vector.tensor_copy(outv[0:B, 1:2], asum[:])
            nc.scalar.dma_start(o_d.ap(), outv[:])

    nc.compile()
    return nc


def prepare(inputs):
    """Host-side prep: rank sections, sort combos by max rank, pair
    short tiles, build device arrays."""
    s = np.asarray(inputs["output"], np.float32)[:, :, 0]
    a1 = np.asarray(inputs["a1"], np.int64)
    a2 = np.asarray(inputs["a2"], np.int64)
    direct = np.asarray(inputs["direct"], np.int64)
    pad = np.asarray(inputs["pad_idx"], np.int64)
    seg = np.asarray(inputs["seg_len"], np.int64)
    NCv = direct.shape[0]
    lane = np.arange(L)[None, :]
    act = lane < seg[:, None]

    padrefs = np.bincount(pad[act], minlength=S)
    dirrefs = np.bincount(direct, minlength=S)
    usage = padrefs + dirrefs
    order = np.argsort(-usage, kind="stable")
    rank = np.empty(S, np.int64)
    rank[order] = np.arange(S)

    # duplicate targets within a combo could push |W entry| past the fp8
    # exact-integer range -> host
    a_ = np.where(act, pad, 2000 + lane)
    tcat = np.concatenate([np.where(direct < S, direct, 3000)[:, None], a_], 1)
    tcat.sort(axis=1)
    host = (tcat[:, 1:] == tcat[:, :-1]).any(1)

    rd = rank[direct]
    rp = np.where(act, rank[pad], 0)
    maxrank = np.maximum(rd, rp.max(1))
    hi_idx = np.flatnonzero((maxrank >= 128) & ~host)
    if hi_idx.size > HI_CAP:
        host[hi_idx[HI_CAP:]] = True

    dev_sorted = np.flatnonzero(~host)
    dev_sorted = dev_sorted[np.argsort(maxrank[dev_sorted], kind="stable")]
    assert dev_sorted.size <= NTOT, "combo overflow"
    core_of = np.full(NCv, -1, np.int64)
    tile_of = np.full(NCv, -1, np.int64)
    col_of = np.full(NCv, -1, np.int64)
    seqpos = np.arange(dev_sorted.size)
    core_of[dev_sorted] = seqpos % CORES
    pos = seqpos // CORES
    tile_of[dev_sorted] = pos // TILE
    col_of[dev_sorted] = pos % TILE

    # per-tile max rank -> pair count (adjacent tiles both <=64 rows;
    # the hi tiles are never paired)
    r_t = np.zeros(T, np.int64)
    mr = np.minimum(maxrank[dev_sorted], 127)
    np.maximum.at(r_t, tile_of[dev_sorted], mr + 1)
    npairs = 0
    while (2 * npairs + 1 < min(HI_TILES) and r_t[2 * npairs] <= 64
           and r_t[2 * npairs + 1] <= 64):
        npairs += 1
    _CACHE["npairs"] = npairs
    nx = T - npairs

    dev = dev_sorted
    pr, pl = np.nonzero(act[dev])
    e_combo = np.concatenate([dev[pr], dev])
    e_row = rank[np.concatenate([pad[dev[pr], pl], direct[dev]])]
    e_val = np.concatenate([np.full(pr.size, -1.0, np.float32),
                            np.full(dev.size, 1.0, np.float32)])
    e_core = core_of[e_combo]
    e_tile = tile_of[e_combo]
    e_col = col_of[e_combo]

    hi_pos_arr = np.full(T, -1, np.int64)
    for i, t_ in enumerate(HI_TILES):
        hi_pos_arr[t_] = i

    WDf = np.zeros((CORES, 128, T, TILE), np.float32)
    lo_e = np.flatnonzero(e_row < 128)
    np.add.at(WDf, (e_core[lo_e], e_row[lo_e], e_tile[lo_e], e_col[lo_e]),
              e_val[lo_e])
    WHf = np.zeros((CORES, 32, N_HI_TILES, TILE), np.float32)
    he = np.flatnonzero(e_row >= 128)
    if he.size:
        np.add.at(WHf, (e_core[he], e_row[he] - 128,
                        hi_pos_arr[e_tile[he]], e_col[he]), e_val[he])
    WD8 = WDf.astype(ml_dtypes.float8_e4m3)
    # transfer-major layout: pairs stack two tiles at partition floors
    WDX = np.zeros((CORES, 128, nx, TILE), ml_dtypes.float8_e4m3)
    for x in range(nx):
        if x < npairs:
            WDX[:, 0:64, x] = WD8[:, 0:64, 2 * x]
            WDX[:, 64:128, x] = WD8[:, 0:64, 2 * x + 1]
        else:
            WDX[:, :, x] = WD8[:, :, npairs + x]
    WD = WDX.view(np.int8).reshape(CORES, 128, nx * TILE)
    WHp = np.zeros((CORES, 128, N_HI_TILES, TILE), ml_dtypes.float8_e4m3)
    WHp[:, 0:32] = WHf.astype(ml_dtypes.float8_e4m3)
    WH = WHp.view(np.int8).reshape(CORES, 128, N_HI_TILES * TILE)

    sTa = np.zeros((160, B), np.float32)
    sTa[rank] = s.T
    SP = np.zeros((128, 3 * B), np.float32)
    SP[:, 0:B] = sTa[0:128]
    SP[0:32, B:2 * B] = sTa[128:160]
    SP[64:128, 2 * B:3 * B] = sTa[0:64]

    r1 = rank[a1]
    r2 = rank[a2]
    WAf = np.zeros((128, 2 * NA), np.float32)
    i_lo1 = np.flatnonzero(r1 < 128)
    np.add.at(WAf, (r1[i_lo1], i_lo1), 1.0)
    i_lo2 = np.flatnonzero(r2 < 128)
    np.add.at(WAf, (r2[i_lo2], i_lo2), -1.0)
    i_hi1 = np.flatnonzero(r1 >= 128)
    np.add.at(WAf, (r1[i_hi1] - 128, NA + i_hi1), 1.0)
    i_hi2 = np.flatnonzero(r2 >= 128)
    np.add.at(WAf, (r2[i_hi2] - 128, NA + i_hi2), -1.0)
    WA = WAf.astype(ml_dtypes.bfloat16).view(np.int16)

    hs = np.flatnonzero(host)
    host_abs = 0.0
    if hs.size:
        m = act[hs].astype(np.float32)
        b2 = np.einsum("bnl,nl->bn", s[:, pad[hs]], m)
        b1 = s[:, direct[hs]]
        host_abs = float(np.abs(b1 - b2).sum())

    in_maps = []
    for c in range(CORES):
        in_maps.append({"SP": SP, "WA": WA, "WD": WD[c], "WH": WH[c]})
    return in_maps, dict(NCv=NCv, host_abs=host_abs, n_host=int(hs.size))


def combine(outs, meta):
    total_abs = meta["host_abs"] + sum(float(outs[i]["outv"][:, 0].sum())
                                       for i in range(CORES))
    mean_a = float(np.exp(outs[0]["outv"][0:B, 1] / NA).mean())
    val = mean_a + total_abs / (B * meta["NCv"])
    return np.asarray(val, dtype=np.float32)


def get_nc():
    npairs = _CACHE.get("npairs", 0)
    key = ("nc", npairs)
    if key not in _CACHE:
        _CACHE[key] = build_nc(npairs)
    return _CACHE[key]


def kernel(**inputs) -> np.ndarray:
    in_maps, meta = prepare(inputs)
    res = run_bass_kernel_spmd(get_nc(), in_maps, core_ids=list(range(CORES)))
    return combine(res.results, meta)
